# revision 64
# baseline (speedup 1.0000x reference)
"""Multi-head causal attention (B=4, S=2048, D=1024, H=16, dk=dv=64) on 8 NeuronCores.

Sharding: core c -> (batch b = c//2, head-group g = c%2 of 8 heads).

v2 design (cost-model driven):
  - All matmul operands bf16 except Q/K, which are stored fp8e4m3 in the
    DoubleRow layout [32 part, 2 dk-half, S]: the scores matmul runs with
    perf_mode=DoubleRow at 0.5 cycles/row (2x over bf16) with effective
    contraction 64.  The 1/sqrt(dk) scale is applied for free via the ACT
    exp's `scale` parameter, so Q/K quantize at their natural range.
  - PV in the [q-part, 65] orientation (output free-dim 65 vs 512): the
    ones-column of V' makes row.. col 64 the softmax denominator.
  - Normalization is a per-partition reciprocal + tensor_scalar multiply
    (denominator is per-q = per-partition in this orientation).
  - attn [q, hv] -> attn^T [hv, q] for the output projection via DMA
    transpose (xbar, 14ns/32x32 tile, no engine time).
  - Output projection accumulates over the 4 head-pair chunks, DVE-copies
    PSUM->SBUF, DMA out.  Host adds the two per-batch partials + the
    constant correction bv @ Wo + bo (bv passes through attention linearly).
  - Emission interleaves projection/output-projection matmuls into the
    attention stream as PE filler while ACT (exp) is the local bottleneck.
"""

import collections
import numpy as np
from contextlib import ExitStack

import concourse.bass as bass
import concourse.mybir as mybir
import concourse.tile as tile
from concourse import bacc, bass_utils

N_HEAD, D_MODEL, D_K, D_V = 16, 1024, 64, 64
BATCH, SEQ = 4, 2048
NCORES = 8
S = SEQ
DM = D_MODEL
HV = 8 * D_V          # 512 local head-value columns per core
KC = DM // 128        # 8 d_model chunks
F32 = mybir.dt.float32
BF16 = mybir.dt.bfloat16
F8 = mybir.dt.float8e4
EXP = mybir.ActivationFunctionType.Exp
MULT = mybir.AluOpType.mult
DR = mybir.MatmulPerfMode.DoubleRow

_CACHED_NC = None


def _build_nc():
    nc = bacc.Bacc("TRN2", target_bir_lowering=False, debug=False)

    xh = nc.dram_tensor("xh", [128, 8 * S], F8, kind="ExternalInput").ap()
    xl = nc.dram_tensor("xl", [128, 8 * S], F8, kind="ExternalInput").ap()
    wqh = nc.dram_tensor("wqh", [128, 8 * HV], F8, kind="ExternalInput").ap()
    wql = nc.dram_tensor("wql", [128, 8 * HV], F8, kind="ExternalInput").ap()
    wkh = nc.dram_tensor("wkh", [128, 8 * HV], F8, kind="ExternalInput").ap()
    wkl = nc.dram_tensor("wkl", [128, 8 * HV], F8, kind="ExternalInput").ap()
    wvh = nc.dram_tensor("wvh", [128, 8 * HV], F8, kind="ExternalInput").ap()
    wvl = nc.dram_tensor("wvl", [128, 8 * HV], F8, kind="ExternalInput").ap()
    wo = nc.dram_tensor("wo", [128, 4 * DM], BF16, kind="ExternalInput").ap()
    bq2 = nc.dram_tensor("bq2", [128, 4], F32, kind="ExternalInput").ap()
    bk2 = nc.dram_tensor("bk2", [128, 4], F32, kind="ExternalInput").ap()
    masks = nc.dram_tensor("masks", [128, 128], BF16, kind="ExternalInput").ap()
    o = nc.dram_tensor("o", [S, DM], BF16, kind="ExternalOutput").ap()

    with tile.TileContext(nc) as tc:
        _build_kernel(tc, nc, xh, xl, wqh, wql, wkh, wkl, wvh, wvl, wo, bq2, bk2, masks, o)
    nc.compile()
    return nc


def _build_kernel(tc, nc, xh, xl, wqh, wql, wkh, wkl, wvh, wvl, wo, bq2, bk2, masks, o, debug=None):
    with ExitStack() as ctx:
        pp = ctx.enter_context(tc.tile_pool(name="persist", bufs=1))
        # PSUM: st 2x2 banks + au 2x1 + pj 2x1 = 8 banks
        ps = ctx.enter_context(tc.tile_pool(name="psum", bufs=2, space="PSUM"))
        ptp = ctx.enter_context(tc.tile_pool(name="ptp", bufs=22))
        rsp = ctx.enter_context(tc.tile_pool(name="rsp", bufs=12))
        osp = ctx.enter_context(tc.tile_pool(name="osp", bufs=5))
        abp = ctx.enter_context(tc.tile_pool(name="abp", bufs=2))

        # ---- persistent tiles ----
        xh_sb = pp.tile([128, 8 * S], F8, name="xh_sb", tag="xh_sb")
        xl_sb = pp.tile([128, 8 * S], F8, name="xl_sb", tag="xl_sb")
        wqh_sb = pp.tile([128, 8 * HV], F8, name="wqh_sb", tag="wqh_sb")
        wql_sb = pp.tile([128, 8 * HV], F8, name="wql_sb", tag="wql_sb")
        wkh_sb = pp.tile([128, 8 * HV], F8, name="wkh_sb", tag="wkh_sb")
        wkl_sb = pp.tile([128, 8 * HV], F8, name="wkl_sb", tag="wkl_sb")
        wvh_sb = pp.tile([128, 8 * HV], F8, name="wvh_sb", tag="wvh_sb")
        wvl_sb = pp.tile([128, 8 * HV], F8, name="wvl_sb", tag="wvl_sb")
        wo_sb = pp.tile([128, 4 * DM], BF16, name="wo_sb", tag="wo_sb")
        q2 = [pp.tile([128, 2 * S], F8, name=f"q2_{qd}", tag=f"q2_{qd}") for qd in range(2)]
        k2 = [pp.tile([128, 2 * S], F8, name=f"k2_{qd}", tag=f"k2_{qd}") for qd in range(2)]
        vpr = [pp.tile([128, 8 * 65], BF16, name=f"vp{sc}", tag=f"vp{sc}")
               for sc in range(S // 128)]
        at_sb = [pp.tile([128, S], BF16, name=f"at{p}", tag=f"at{p}") for p in range(4)]
        mask_sb = pp.tile([128, 128], BF16, name="mask_sb", tag="mask_sb")
        bq2_sb = pp.tile([128, 4], F32, name="bq2_sb", tag="bq2_sb")
        bk2_sb = pp.tile([128, 4], F32, name="bk2_sb", tag="bk2_sb")

        # DMA order matters: the sim's DMA device is exclusive, so front-load
        # exactly what A(0)'s first matmuls need (wq hi/lo + x hi/lo s-tile 0).
        def _x4(ap):
            return ap.rearrange("p (cp ko s) -> p cp ko s", cp=4, ko=2)

        def _w4(ap):
            return ap.rearrange("p (cp ko c) -> p cp ko c", cp=4, ko=2)

        nc.sync.dma_start(out=_w4(wqh_sb[:])[:, 0:2, :, :], in_=_w4(wqh)[:, 0:2, :, :])
        nc.sync.dma_start(out=_x4(xh_sb[:])[:, 0:2, :, 0:512], in_=_x4(xh)[:, 0:2, :, 0:512])
        nc.sync.dma_start(out=_w4(wqh_sb[:])[:, 2:4, :, :], in_=_w4(wqh)[:, 2:4, :, :])
        nc.sync.dma_start(out=_x4(xh_sb[:])[:, 2:4, :, 0:512], in_=_x4(xh)[:, 2:4, :, 0:512])
        nc.sync.dma_start(out=wql_sb[:], in_=wql)
        nc.sync.dma_start(out=_x4(xl_sb[:])[:, :, :, 0:512], in_=_x4(xl)[:, :, :, 0:512])
        nc.sync.dma_start(out=bq2_sb[:], in_=bq2)
        nc.sync.dma_start(out=bk2_sb[:], in_=bk2)
        nc.sync.dma_start(out=wkh_sb[:], in_=wkh)
        nc.sync.dma_start(out=wkl_sb[:], in_=wkl)
        nc.sync.dma_start(out=mask_sb[:], in_=masks)
        nc.sync.dma_start(out=wvh_sb[:], in_=wvh)
        nc.sync.dma_start(out=wvl_sb[:], in_=wvl)
        for t in range(1, 4):
            nc.sync.dma_start(
                out=_x4(xh_sb[:])[:, :, :, t * 512:(t + 1) * 512],
                in_=_x4(xh)[:, :, :, t * 512:(t + 1) * 512],
            )
            nc.sync.dma_start(
                out=_x4(xl_sb[:])[:, :, :, t * 512:(t + 1) * 512],
                in_=_x4(xl)[:, :, :, t * 512:(t + 1) * 512],
            )
        nc.sync.dma_start(out=wo_sb[:], in_=wo)

        # ---- emission helpers ----
        # A projections: 3-term fp8 DoubleRow split at common scale 32:
        #   32*(x@W) ~= xh@Wh + xl@Wh + xh@Wl
        # with xh=fp8(x), xl=fp8(x-xh), Wh=fp8(32W), Wl=fp8(32W-Wh).
        def emit_q_proj(wh, wl, bias_sb, dst, t, qd, jj, nm):
            pq = ps.tile([128, 512], F32, name=f"p{nm}_{t}_{qd}_{jj}", tag="pj")
            c0 = qd * 256 + jj * 128
            n = 0
            for xop, wop in ((xh_sb, wh), (xh_sb, wl), (xl_sb, wh)):
                for cp in range(4):
                    nc.tensor.matmul(
                        pq[:],
                        lhsT=_w4(wop[:])[:, cp, :, c0:c0 + 128],
                        rhs=_x4(xop[:])[:, cp, :, t * 512:(t + 1) * 512],
                        start=(n == 0), stop=(n == 11),
                        perf_mode=DR,
                    )
                    n += 1
            with nc.allow_low_precision(reason="fp8 Q/K storage is the design"):
                nc.vector.tensor_scalar(
                    out=dst[qd][:, jj * S + t * 512: jj * S + (t + 1) * 512],
                    in0=pq[:],
                    scalar1=1.0 / 32.0,
                    scalar2=bias_sb[:, qd * 2 + jj: qd * 2 + jj + 1],
                    op0=MULT,
                    op1=mybir.AluOpType.add,
                )

        def emit_v_proj(sc):
            pv = ps.tile([128, 512], F32, name=f"pv_{sc}", tag="pj")
            n = 0
            for xop, wop in ((xh_sb, wvh_sb), (xl_sb, wvh_sb), (xh_sb, wvl_sb)):
                for cp in range(4):
                    nc.tensor.matmul(
                        pv[:],
                        lhsT=_x4(xop[:])[:, cp, :, sc * 128:(sc + 1) * 128],
                        rhs=_w4(wop[:])[:, cp, :, :],
                        start=(n == 0), stop=(n == 11),
                        perf_mode=DR,
                    )
                    n += 1
            with nc.allow_low_precision(reason="bf16 V storage is the design"):
                # GPSIMD cannot read PSUM -> this copy+descale lives on DVE
                nc.vector.tensor_scalar(
                    out=vpr[sc][:].rearrange("p (h c) -> p h c", h=8)[:, :, 0:64],
                    in0=pv[:].rearrange("p (h c) -> p h c", h=8),
                    scalar1=1.0 / 32.0,
                    scalar2=None,
                    op0=MULT,
                )
            nc.gpsimd.memset(
                vpr[sc][:].rearrange("p (h c) -> p h c", h=8)[:, :, 64:65], 1.0
            )

        def a_tile_closures(t):
            out = []
            for jj in range(2):
                out.append((t, lambda t=t, jj=jj: emit_q_proj(wqh_sb, wql_sb, bq2_sb, q2, t, 0, jj, "q")))
            for jj in range(2):
                out.append((t, lambda t=t, jj=jj: emit_q_proj(wkh_sb, wkl_sb, bk2_sb, k2, t, 0, jj, "k")))
            for sc in range(4 * t, 4 * t + 4):
                out.append((t + 0.3, lambda sc=sc: emit_v_proj(sc)))
            for jj in range(2):
                out.append((t + 0.5, lambda t=t, jj=jj: emit_q_proj(wqh_sb, wql_sb, bq2_sb, q2, t, 1, jj, "q")))
            for jj in range(2):
                out.append((t + 0.5, lambda t=t, jj=jj: emit_q_proj(wkh_sb, wkl_sb, bk2_sb, k2, t, 1, jj, "k")))
            return out

        osb_cur = {}

        def emit_c(jb, sc, m):
            # batch: one osb tile and one output DMA per s-chunk
            if sc not in osb_cur:
                osb_cur[sc] = osp.tile([128, DM], BF16, name=f"osb_{sc}", tag="osb")
            osb = osb_cur[sc]
            if jb == 3:
                tag = ("pj", "st", "au")[(2 * sc + m) % 3]
            else:
                tag = "pj"
            shape = [128, 1024] if tag == "st" else [128, 512]
            oc_t = ps.tile(shape, F32, name=f"oc_{sc}_{m}", tag=tag)
            oc = oc_t if tag != "st" else oc_t
            for p in range(4):
                nc.tensor.matmul(
                    oc[:, 0:512],
                    lhsT=at_sb[p][:, sc * 128:(sc + 1) * 128],
                    rhs=wo_sb[:, p * DM + m * 512: p * DM + (m + 1) * 512],
                    start=(p == 0), stop=(p == 3),
                )
            with nc.allow_low_precision(reason="bf16 partials are the design"):
                nc.vector.tensor_copy(out=osb[:, m * 512:(m + 1) * 512], in_=oc[:, 0:512])
            if m == 1:
                nc.sync.dma_start(out=o[sc * 128:(sc + 1) * 128, :], in_=osb[:])
                del osb_cur[sc]

        fillq = collections.deque()   # A-projection closures, deadline-tagged
        fillq_c = collections.deque()  # output-projection closures, late-popped

        def pop_fill(n=1, allow_c=False):
            for _ in range(n):
                if fillq:
                    fillq.popleft()[1]()
                elif allow_c and fillq_c:
                    fillq_c.popleft()()
                else:
                    return

        def drain_fill(deadline):
            while fillq and fillq[0][0] <= deadline:
                fillq.popleft()[1]()

        for t in (1, 2, 3):
            for dl, fn in a_tile_closures(t):
                fillq.append((dl, fn))

        # ---- B-phase emission helpers ----
        def emit_scores_pc(h, j, pc, pts):
            qd, hh = divmod(h, 4)
            r0, r1 = 32 * hh, 32 * hh + 32
            vp = max(0, 256 * pc - 512 * j)
            st = ps.tile([128, 1024], F32, name=f"st_{h}_{j}_{pc}", tag="st")
            k2v = k2[qd][r0:r1, :].rearrange("p (jh s) -> p jh s", jh=2)
            q2v = q2[qd][r0:r1, :].rearrange("p (jh s) -> p jh s", jh=2)
            for u01 in range(2):
                kc = 2 * pc + u01
                nc.tensor.matmul(
                    st[:, u01 * 512 + vp:(u01 + 1) * 512],
                    lhsT=k2v[:, :, kc * 128:(kc + 1) * 128],
                    rhs=q2v[:, :, j * 512 + vp:(j + 1) * 512],
                    start=True, stop=True,
                    perf_mode=DR,
                    tile_position=(r0, 0),
                )
            pt = ptp.tile([128, 1024], BF16, name=f"pt_{h}_{j}_{pc}", tag="pt")
            st3 = st[:].rearrange("p (u c) -> p u c", u=2)
            pt3 = pt[:].rearrange("p (u c) -> p u c", u=2)
            with nc.allow_low_precision(reason="bf16 P is the design"):
                nc.scalar.activation(
                    pt3[:, :, vp:512], st3[:, :, vp:512], EXP, scale=0.125
                )
            for u01 in range(2):
                i = 2 * pc + u01 - 4 * j
                if i >= 0:
                    c0 = u01 * 512 + 128 * i
                    with nc.allow_low_precision(reason="bf16 P is the design"):
                        nc.vector.tensor_tensor(
                            out=pt[:, c0:c0 + 128],
                            in0=pt[:, c0:c0 + 128],
                            in1=mask_sb[:, 0:128],
                            op=MULT,
                        )
            pts[pc] = pt

        def emit_pv_head(h, j, au, pts):
            # u-major: each au region's accumulation chain is contiguous.
            # (a start_tensor_calc mid-bank clears has_written bank-wide, so
            # interleaving accumulation groups in one bank corrupts them)
            for u in range(4):
                for kc in range(4 * j + u + 1):
                    pc, u01 = divmod(kc, 2)
                    nc.tensor.matmul(
                        au[:, u * 128: u * 128 + 65],
                        lhsT=pts[pc][:, u01 * 512 + u * 128: u01 * 512 + (u + 1) * 128],
                        rhs=vpr[kc][:, h * 65:(h + 1) * 65],
                        start=(kc == 0), stop=(kc == 4 * j + u),
                        skip_group_check=True,
                    )

        def emit_norm(h, j, au, attn_t):
            hp = h % 2
            for u in range(4):
                r_sb = rsp.tile([128, 1], F32, name=f"r_{h}_{j}_{u}", tag="r")
                nc.vector.reciprocal(
                    out=r_sb[:], in_=au[:, u * 128 + 64: u * 128 + 65]
                )
                with nc.allow_low_precision(reason="bf16 attn is the design"):
                    nc.vector.tensor_scalar_mul(
                        out=attn_t[:, u * 128 + hp * 64: u * 128 + (hp + 1) * 64],
                        in0=au[:, u * 128: u * 128 + 64],
                        scalar1=r_sb[:, 0:1],
                    )

        def emit_transpose(pair, j, attn_t):
            nc.sync.dma_start_transpose(
                out=at_sb[pair][:, j * 512:(j + 1) * 512].rearrange(
                    "p (u q) -> p u q", u=4
                ),
                in_=attn_t[:],
            )

        def queue_c(j):
            for sc in range(4 * j, 4 * j + 4):
                for m in range(2):
                    fillq_c.append(lambda jb=j, sc=sc, m=m: emit_c(jb, sc, m))

        # ---- j = 0, interleaved with phase A tile 0 ----
        # Q/K quad-0 projections first, then each head's scores slots between
        # the remaining A(0) groups so ACT (exp) starts as early as possible.
        for fn in (
            lambda: emit_q_proj(wqh_sb, wql_sb, bq2_sb, q2, 0, 0, 0, "q"),
            lambda: emit_q_proj(wqh_sb, wql_sb, bq2_sb, q2, 0, 0, 1, "q"),
            lambda: emit_q_proj(wkh_sb, wkl_sb, bk2_sb, k2, 0, 0, 0, "k"),
            lambda: emit_q_proj(wkh_sb, wkl_sb, bk2_sb, k2, 0, 0, 1, "k"),
        ):
            fn()
        inter = [
            lambda: emit_q_proj(wqh_sb, wql_sb, bq2_sb, q2, 0, 1, 0, "q"),
            lambda: emit_q_proj(wqh_sb, wql_sb, bq2_sb, q2, 0, 1, 1, "q"),
            lambda: emit_q_proj(wkh_sb, wkl_sb, bk2_sb, k2, 0, 1, 0, "k"),
            lambda: emit_q_proj(wkh_sb, wkl_sb, bk2_sb, k2, 0, 1, 1, "k"),
            lambda: emit_v_proj(0),
            lambda: emit_v_proj(1),
            lambda: emit_v_proj(2),
            lambda: emit_v_proj(3),
        ]
        j0 = {}
        for h in range(8):
            pts = {}
            for pc in range(2):
                emit_scores_pc(h, 0, pc, pts)
            j0[h] = pts
            inter.pop(0)()
        attn_pair = {}
        for h in range(8):
            pts = j0[h]
            au = ps.tile([128, 512], F32, name=f"au_{h}_0", tag="au")
            pair, hp = divmod(h, 2)
            if hp == 0:
                attn_pair[pair] = abp.tile(
                    [128, 512], BF16, name=f"attn_{pair}_0", tag=f"attn_{pair}_0"
                )
            emit_pv_head(h, 0, au, pts)
            emit_norm(h, 0, au, attn_pair[pair])
            if hp == 1:
                emit_transpose(pair, 0, attn_pair[pair])
                if debug is not None and pair == 0:
                    nc.sync.dma_start(out=debug["dattn"], in_=attn_pair[pair][:])
            pop_fill(1)
        queue_c(0)

        # ---- j = 1..3 + interleaved filler ----
        slot = [0]
        for j in range(1, 4):
            drain_fill(j)
            attn_pair = {}
            for h in range(8):
                if h == 4:
                    drain_fill(j + 0.5)
                pair, hp = divmod(h, 2)
                au = ps.tile([128, 512], F32, name=f"au_{h}_{j}", tag="au")
                if hp == 0:
                    attn_pair[pair] = abp.tile(
                        [128, 512], BF16, name=f"attn_{pair}_{j}",
                        tag=f"attn_{pair}_{j}",
                    )
                attn_t = attn_pair[pair]
                pts = {}
                for pc in range(2 * j + 2):
                    emit_scores_pc(h, j, pc, pts)
                    slot[0] += 1
                    if slot[0] % 4 == 0:
                        pop_fill(1, allow_c=(j >= 3))
                drain_fill(j + 0.3)
                emit_pv_head(h, j, au, pts)
                emit_norm(h, j, au, attn_t)
                if hp == 1:
                    emit_transpose(pair, j, attn_t)
                pts.clear()
            queue_c(j)

        # ---- tail: remaining filler (C blocks) ----
        drain_fill(99)
        while fillq_c:
            fillq_c.popleft()()

        if debug is not None:
            nc.sync.dma_start(out=debug["dq2"], in_=q2[0][:])
            nc.sync.dma_start(out=debug["dk2"], in_=k2[0][:])
            nc.sync.dma_start(out=debug["dvp"], in_=vpr[0][:])
            nc.sync.dma_start(out=debug["dat"], in_=at_sb[0][:])


def _masks_np():
    r = np.arange(128)[:, None]
    c = np.arange(128)[None, :]
    return (c >= r).astype(np.float32)


def _qk_perm(g):
    """Column permutation for the DoubleRow Q/K layout.

    dst col (qd*256 + jj*128 + hh*32 + d) <- src col 64*(8g + 4qd + hh) + 32jj + d.
    Projection-output partition hh*32+d then holds head (8g+4qd+hh), dk 32jj+d.
    """
    perm = np.empty(512, np.int64)
    for qd in range(2):
        for jj in range(2):
            for hh in range(4):
                src = 64 * (8 * g + 4 * qd + hh) + 32 * jj
                dst = qd * 256 + jj * 128 + hh * 32
                perm[dst:dst + 32] = np.arange(src, src + 32)
    return perm


def make_in_maps(input, Wq, bq, Wk, bk, Wv, Wo):
    import ml_dtypes

    bf16 = ml_dtypes.bfloat16
    f8 = ml_dtypes.float8_e4m3fn

    def split8(M):
        hi = np.asarray(M, f8).astype(np.float32)
        lo = np.asarray(M - hi, f8)
        return hi.astype(f8), lo

    def pack(M):
        # [1024, C] -> [128, 4*2*C]: out[p, (cp, ko, c)] = M[256 cp + 128 ko + p, c]
        C = M.shape[1]
        return np.ascontiguousarray(
            M.reshape(4, 2, 128, C).transpose(2, 0, 1, 3).reshape(128, 8 * C)
        )

    masks = _masks_np().astype(bf16)
    input = np.asarray(input, np.float32)
    Wq, bq = np.asarray(Wq, np.float32), np.asarray(bq, np.float32)
    Wk, bk = np.asarray(Wk, np.float32), np.asarray(bk, np.float32)
    Wv, Wo = np.asarray(Wv, np.float32), np.asarray(Wo, np.float32)
    in_maps = []
    for c in range(NCORES):
        b, g = divmod(c, 2)
        perm = _qk_perm(g)
        bq2 = bq[perm].reshape(2, 2, 128).transpose(2, 0, 1).reshape(128, 4)
        bk2 = bk[perm].reshape(2, 2, 128).transpose(2, 0, 1).reshape(128, 4)
        wo_r = (
            Wo[g * HV:(g + 1) * HV, :]
            .reshape(4, 128, DM)
            .transpose(1, 0, 2)
            .reshape(128, 4 * DM)
        )
        xh, xl = split8(input[b].T)
        wqh, wql = split8(32.0 * Wq[:, perm])
        wkh, wkl = split8(32.0 * Wk[:, perm])
        wvh, wvl = split8(32.0 * Wv[:, g * HV:(g + 1) * HV])
        in_maps.append(
            {
                "xh": pack(xh), "xl": pack(xl),
                "wqh": pack(wqh), "wql": pack(wql),
                "wkh": pack(wkh), "wkl": pack(wkl),
                "wvh": pack(wvh), "wvl": pack(wvl),
                "wo": np.ascontiguousarray(wo_r).astype(bf16),
                "bq2": np.ascontiguousarray(bq2),
                "bk2": np.ascontiguousarray(bk2),
                "masks": masks,
            }
        )
    return in_maps


def _numpy_fallback(input, attn_mask, Wq, bq, Wk, bk, Wv, bv, Wo, bo):
    """Host fallback for non-causal masks (should not trigger in practice)."""
    x = np.asarray(input, np.float32)
    mask = np.asarray(attn_mask)
    B, S_, _ = x.shape
    scale = np.float32(1.0 / np.sqrt(D_K))
    out = np.empty((B, S_, D_MODEL), np.float32)
    for b in range(B):
        q = (x[b] @ Wq + bq).reshape(S_, N_HEAD, D_K)
        k = (x[b] @ Wk + bk).reshape(S_, N_HEAD, D_K)
        v = (x[b] @ Wv + bv).reshape(S_, N_HEAD, D_V)
        attn = np.empty((S_, N_HEAD, D_V), np.float32)
        for h in range(N_HEAD):
            score = (q[:, h] @ k[:, h].T) * scale
            score = np.where(mask, -np.inf, score)
            score -= score.max(axis=-1, keepdims=True)
            p = np.exp(score)
            p /= p.sum(axis=-1, keepdims=True)
            attn[:, h] = p @ v[:, h]
        out[b] = attn.reshape(S_, N_HEAD * D_V) @ Wo + bo
    return out


_CACHED_RUNNER = None


def _make_runner(nc):
    """Build the shard_map-jitted PJRT executor once; reuse across calls."""
    import jax
    from jax.sharding import Mesh, PartitionSpec
    from jax.experimental.shard_map import shard_map
    from concourse import bass2jax

    bass2jax.install_neuronx_cc_hook()
    partition_name = nc.partition_id_tensor.name if nc.partition_id_tensor else None
    in_names, out_names, out_avals, zero_outs = [], [], [], []
    for alloc in nc.m.functions[0].allocations:
        if not isinstance(alloc, mybir.MemoryLocationSet):
            continue
        name = alloc.memorylocations[0].name
        if alloc.kind == "ExternalInput":
            if name != partition_name:
                in_names.append(name)
        elif alloc.kind == "ExternalOutput":
            out_names.append(name)
            shape = tuple(alloc.tensor_shape)
            dtype = mybir.dt.np(alloc.dtype)
            out_avals.append(jax.core.ShapedArray(shape, dtype))
            zero_outs.append(np.zeros(shape, dtype))
    n_params = len(in_names)
    n_outs = len(out_avals)
    all_in_names = list(in_names) + list(out_names)
    if partition_name is not None:
        all_in_names.append(partition_name)

    def _body(*args):
        operands = list(args)
        if partition_name is not None:
            operands.append(bass2jax.partition_id_tensor())
        outs = bass2jax._bass_exec_p.bind(
            *operands,
            out_avals=tuple(out_avals),
            in_names=tuple(all_in_names),
            out_names=tuple(out_names),
            lowering_input_output_aliases=(),
            sim_require_finite=True,
            sim_require_nnan=True,
            nc=nc,
        )
        return tuple(outs)

    devices = jax.devices()[:NCORES]
    mesh = Mesh(np.asarray(devices), ("core",))
    sharded = jax.jit(
        shard_map(
            _body,
            mesh=mesh,
            in_specs=(PartitionSpec("core"),) * (n_params + n_outs),
            out_specs=(PartitionSpec("core"),) * n_outs,
            check_rep=False,
        ),
        donate_argnums=tuple(range(n_params, n_params + n_outs)),
        keep_unused=True,
    )

    def run(in_maps):
        concat_in = [
            np.concatenate(
                [np.asarray(in_maps[c][nm]) for c in range(NCORES)], axis=0
            )
            for nm in in_names
        ]
        concat_zeros = [
            np.zeros((NCORES * z.shape[0], *z.shape[1:]), z.dtype) for z in zero_outs
        ]
        out_arrs = sharded(*concat_in, *concat_zeros)
        return [
            {
                nm: np.asarray(out_arrs[i]).reshape(NCORES, *out_avals[i].shape)[c]
                for i, nm in enumerate(out_names)
            }
            for c in range(NCORES)
        ]

    return run


def kernel(input, attn_mask, Wq, bq, Wk, bk, Wv, bv, Wo, bo):
    causal = np.triu(np.ones((SEQ, SEQ), bool), k=1)
    if not np.array_equal(np.asarray(attn_mask), causal):
        return _numpy_fallback(input, attn_mask, Wq, bq, Wk, bk, Wv, bv, Wo, bo)

    global _CACHED_NC, _CACHED_RUNNER
    if _CACHED_NC is None:
        _CACHED_NC = _build_nc()

    in_maps = make_in_maps(input, Wq, bq, Wk, bk, Wv, Wo)
    try:
        if _CACHED_RUNNER is None:
            _CACHED_RUNNER = _make_runner(_CACHED_NC)
        outs = _CACHED_RUNNER(in_maps)
    except Exception:
        _CACHED_RUNNER = None
        outs = bass_utils.run_bass_kernel_spmd(
            _CACHED_NC, in_maps, core_ids=list(range(NCORES))
        ).results

    corr = (
        np.asarray(bv, np.float32) @ np.asarray(Wo, np.float32)
        + np.asarray(bo, np.float32)
    ).astype(np.float32)
    out = np.empty((BATCH, SEQ, D_MODEL), np.float32)
    for b in range(BATCH):
        out[b] = (
            outs[2 * b]["o"].astype(np.float32)
            + outs[2 * b + 1]["o"].astype(np.float32)
            + corr[None, :]
        )
    return out
